# revision 1
# baseline (speedup 1.0000x reference)
"""GCN layer (GCNConv + PReLU) on TRN2, SPMD across 8 NeuronCores.

Problem: out = PReLU(A_hat @ (x @ W) + b), A_hat = D^-1/2 (A+I) D^-1/2,
x: [100000, 128] f32, edge_index: [2, 1600000] int, W: [128,128], b,
prelu_a: [128].

Strategy (aggregation commutes with the linear map): out = PReLU((A_hat@x)@W+b).
Nodes are split into 8 contiguous ranges of 12500 (one per core). Edges
(+self loops) are partitioned by dst core and sorted by dst, so scatter-add is
core-local. Each core keeps the full x table in its DRAM and:
  - gathers x[src] rows with indirect DMA (the dominant, memory-bound step)
  - builds H[e, j] = norm_e * (dstloc_e == j) in one fused DVE tensor_scalar
  - PE accumulates accT[ch, node] += rows.T @ H per 128-node window in PSUM
  - per window: z = accT.T @ W + b (PE, rank-1 trick for b), PReLU (DVE), DMA
No collectives. Host work is index/sharding prep only (sort, degree counts,
per-edge norm scalars, padding to 128-edge tiles uniform across cores).

Implementation notes for this toolchain:
  - the program must be built as bacc.Bacc and .compile()d so bacc's
    generate_event_semaphores pass splits multi-sem waits (walrus codegen
    accepts at most ~2 sync commands per instruction).
  - HW indirect DMA honors ONE dynamic offset per partition (extra offset
    columns are ignored; it streams consecutive rows), so each gather moves
    exactly 128 rows (k_gather = 1). CoreSim models multi-offset gathers,
    hardware does not.
"""

import math

import numpy as np

import concourse.bacc as bacc
import concourse.mybir as mybir
import concourse.tile as tile
from concourse.bass import IndirectOffsetOnAxis
from concourse.bass_utils import run_bass_kernel_spmd

P = 128
N_CORES = 8
N_NODES = 100000
K_GATHER = 1

F32 = mybir.dt.float32
I32 = mybir.dt.int32


def _build_program(n_table_rows, n_out_rows, win_tiles, k_gather=K_GATHER):
    n_win = len(win_tiles)
    assert n_win == math.ceil(n_out_rows / P)
    T = int(sum(win_tiles))  # total 128-edge tiles

    tile_win = np.repeat(np.arange(n_win), win_tiles)
    win_start = np.zeros(n_win, dtype=np.int64)
    np.cumsum(win_tiles[:-1], out=win_start[1:])

    nc = bacc.Bacc("TRN2", target_bir_lowering=False)
    x = nc.declare_dram_parameter("x", [n_table_rows, P], F32, isOutput=False)
    # meta columns: [0:T) dstloc f32, [T:2T) norm f32, [2T:3T) src int32 bits
    meta = nc.declare_dram_parameter("meta", [P, 3 * T], F32, isOutput=False)
    w_p = nc.declare_dram_parameter("W", [P, P], F32, isOutput=False)
    b_p = nc.declare_dram_parameter("b", [1, P], F32, isOutput=False)
    a_p = nc.declare_dram_parameter("prelu_bcast", [P, P], F32, isOutput=False)
    iota_p = nc.declare_dram_parameter("iota", [P, P], F32, isOutput=False)
    y = nc.declare_dram_parameter("y", [n_out_rows, P], F32, isOutput=True)

    with tile.TileContext(nc) as tc:
        with (
            tc.tile_pool(name="meta", bufs=1) as meta_pool,
            tc.tile_pool(name="const", bufs=1) as const_pool,
            tc.tile_pool(name="rows", bufs=24) as rows_pool,
            tc.tile_pool(name="h", bufs=16) as h_pool,
            tc.tile_pool(name="epi", bufs=3) as epi_pool,
            tc.tile_pool(name="psum", bufs=2, space="PSUM") as psum_pool,
        ):
            meta_t = meta_pool.tile([P, 3 * T], F32, tag="meta")
            nc.sync.dma_start(out=meta_t[:], in_=meta[:, :])

            w_t = const_pool.tile([P, P], F32, tag="W")
            a_t = const_pool.tile([P, P], F32, tag="prelu")
            iota_t = const_pool.tile([P, P], F32, tag="iota")
            b_t = const_pool.tile([1, P], F32, tag="b")
            ones_t = const_pool.tile([1, P], F32, tag="ones")
            nc.sync.dma_start(out=w_t[:], in_=w_p[:, :])
            nc.sync.dma_start(out=a_t[:], in_=a_p[:, :])
            nc.sync.dma_start(out=iota_t[:], in_=iota_p[:, :])
            nc.sync.dma_start(out=b_t[:], in_=b_p[:, :])
            nc.vector.memset(ones_t[:], 1.0)

            accT = None

            def epilogue(w, accT_tile):
                r0 = w * P
                nr = min(P, n_out_rows - r0)
                accT_sb = epi_pool.tile([P, P], F32, tag="accT_sb")
                nc.vector.tensor_copy(out=accT_sb[:], in_=accT_tile[:])
                outp = psum_pool.tile([P, P], F32, tag="outp")
                nc.tensor.matmul(
                    out=outp[:], lhsT=accT_sb[:], rhs=w_t[:], start=True, stop=False
                )
                nc.tensor.matmul(
                    out=outp[:], lhsT=ones_t[:], rhs=b_t[:], start=False, stop=True
                )
                zpos = epi_pool.tile([P, P], F32, tag="zpos")
                nc.vector.tensor_scalar(
                    out=zpos[:],
                    in0=outp[:],
                    scalar1=0.0,
                    scalar2=None,
                    op0=mybir.AluOpType.max,
                )
                zneg = epi_pool.tile([P, P], F32, tag="zneg")
                nc.vector.tensor_tensor(
                    out=zneg[:], in0=outp[:], in1=zpos[:], op=mybir.AluOpType.subtract
                )
                zs = epi_pool.tile([P, P], F32, tag="zs")
                nc.vector.tensor_tensor(
                    out=zs[:], in0=zneg[:], in1=a_t[:], op=mybir.AluOpType.mult
                )
                out_sb = epi_pool.tile([P, P], F32, tag="out_sb")
                nc.vector.tensor_tensor(
                    out=out_sb[:], in0=zpos[:], in1=zs[:], op=mybir.AluOpType.add
                )
                nc.sync.dma_start(out=y[r0 : r0 + nr, :], in_=out_sb[:nr, :])

            for c0 in range(0, T, k_gather):
                k = min(k_gather, T - c0)
                rows = rows_pool.tile([P, k_gather * P], F32, tag="rows")
                nc.gpsimd.indirect_dma_start(
                    out=rows[:, : k * P],
                    out_offset=None,
                    in_=x[:, :],
                    in_offset=IndirectOffsetOnAxis(
                        ap=meta_t[:, 2 * T + c0 : 2 * T + c0 + k].bitcast(I32),
                        axis=0,
                    ),
                )
                for j in range(k):
                    t = c0 + j
                    w = int(tile_win[t])
                    first = t == int(win_start[w])
                    last = t == int(win_start[w]) + int(win_tiles[w]) - 1
                    if first:
                        accT = psum_pool.tile([P, P], F32, tag="accT")
                    h_t = h_pool.tile([P, P], F32, tag="h")
                    nc.vector.tensor_scalar(
                        out=h_t[:],
                        in0=iota_t[:],
                        scalar1=meta_t[:, t : t + 1],
                        scalar2=meta_t[:, T + t : T + t + 1],
                        op0=mybir.AluOpType.is_equal,
                        op1=mybir.AluOpType.mult,
                    )
                    nc.tensor.matmul(
                        out=accT[:],
                        lhsT=rows[:, j * P : (j + 1) * P],
                        rhs=h_t[:],
                        start=first,
                        stop=last,
                    )
                    if last:
                        epilogue(w, accT)
    nc.compile()
    return nc


def _preprocess(x, edge_index, n_cores=N_CORES):
    N = x.shape[0]
    src = np.asarray(edge_index[0], dtype=np.int64)
    dst = np.asarray(edge_index[1], dtype=np.int64)
    loop = np.arange(N, dtype=np.int64)
    src = np.concatenate([src, loop])
    dst = np.concatenate([dst, loop])
    deg = np.bincount(dst, minlength=N)
    dinv = (1.0 / np.sqrt(deg.astype(np.float64))).astype(np.float32)
    norm = dinv[src] * dinv[dst]

    rows_per_core = N // n_cores
    n_win = math.ceil(rows_per_core / P)

    order = np.argsort(dst, kind="stable")
    src_s = src[order].astype(np.int32)
    dst_s = dst[order]
    norm_s = norm[order]

    core_id = dst_s // rows_per_core
    local = dst_s - core_id * rows_per_core
    win = local // P
    dstloc = (local % P).astype(np.float32)

    group = core_id * n_win + win  # non-decreasing (edges sorted by dst)
    counts = np.bincount(group, minlength=n_cores * n_win).reshape(n_cores, n_win)
    # Deal each core's windows to slots in count-sorted order so the SPMD
    # max-across-cores tile count per slot shrinks toward the per-core ideal.
    # The short last window (rows_per_core % 128) stays pinned at the last slot.
    perm = np.empty((n_cores, n_win), dtype=np.int64)  # perm[c, slot] = window
    for c in range(n_cores):
        perm[c, : n_win - 1] = np.argsort(-counts[c, : n_win - 1], kind="stable")
        perm[c, n_win - 1] = n_win - 1
    inv_perm = np.empty_like(perm)  # inv_perm[c, window] = slot
    np.put_along_axis(inv_perm, perm, np.arange(n_win)[None, :], axis=1)
    slot_counts = np.take_along_axis(counts, perm, axis=1)
    win_tiles = np.maximum(1, -(-slot_counts.max(axis=0) // P))
    T = int(win_tiles.sum())
    win_tile_start = np.zeros(n_win, dtype=np.int64)
    np.cumsum(win_tiles[:-1], out=win_tile_start[1:])

    group_start = np.zeros(n_cores * n_win, dtype=np.int64)
    np.cumsum(counts.ravel()[:-1], out=group_start[1:])
    rank = np.arange(len(dst_s)) - group_start[group]
    edge_slot = inv_perm[core_id, win]
    slot = win_tile_start[edge_slot] * P + rank

    metas = []
    for c in range(n_cores):
        m = core_id == c
        dstloc_pad = np.zeros(T * P, dtype=np.float32)
        norm_pad = np.zeros(T * P, dtype=np.float32)
        src_pad = np.zeros(T * P, dtype=np.int32)
        s = slot[m]
        dstloc_pad[s] = dstloc[m]
        norm_pad[s] = norm_s[m]
        src_pad[s] = src_s[m]
        # [P, 3T]: tile t lives in column t; SBUF partition p = edge t*128+p
        meta = np.empty((P, 3 * T), dtype=np.float32)
        meta[:, 0:T] = dstloc_pad.reshape(T, P).T
        meta[:, T : 2 * T] = norm_pad.reshape(T, P).T
        meta[:, 2 * T : 3 * T] = src_pad.reshape(T, P).T.view(np.float32)
        metas.append({"meta": np.ascontiguousarray(meta)})
    return metas, [int(t) for t in win_tiles], rows_per_core, perm


def _make_in_maps(x, W, b, prelu_a, metas):
    consts = {
        "x": np.ascontiguousarray(np.asarray(x, dtype=np.float32)),
        "W": np.ascontiguousarray(np.asarray(W, dtype=np.float32)),
        "b": np.asarray(b, dtype=np.float32).reshape(1, P),
        "prelu_bcast": np.ascontiguousarray(
            np.tile(np.asarray(prelu_a, dtype=np.float32), (P, 1))
        ),
        "iota": np.tile(np.arange(P, dtype=np.float32), (P, 1)),
    }
    return [{**consts, **metas[c]} for c in range(N_CORES)]


def _unscramble(y_slot_order, perm, rows_per_core):
    """y rows are in per-core slot order; map slot s -> window perm[c, s]."""
    n_win = perm.shape[1]
    out = np.empty_like(y_slot_order)
    for c in range(perm.shape[0]):
        yc = y_slot_order[c * rows_per_core : (c + 1) * rows_per_core]
        oc = out[c * rows_per_core : (c + 1) * rows_per_core]
        for s in range(n_win):
            w = int(perm[c, s])
            nr = min(P, rows_per_core - w * P)
            oc[w * P : w * P + nr] = yc[s * P : s * P + nr]
    return out


def build_all(x, edge_index, W, b, prelu_a):
    """Preprocess + build. Returns (nc, in_maps, rows_per_core, unscramble)."""
    metas, win_tiles, rows_per_core, perm = _preprocess(x, edge_index)
    nc = _build_program(
        n_table_rows=x.shape[0], n_out_rows=rows_per_core, win_tiles=win_tiles
    )
    unscramble = lambda y: _unscramble(y, perm, rows_per_core)
    return nc, _make_in_maps(x, W, b, prelu_a, metas), rows_per_core, unscramble


def kernel(x, edge_index, W, b, prelu_a):
    nc, in_maps, _, unscramble = build_all(x, edge_index, W, b, prelu_a)
    res = run_bass_kernel_spmd(nc, in_maps, core_ids=list(range(N_CORES)))
    y = np.concatenate([res.results[c]["y"] for c in range(N_CORES)], axis=0)
    return unscramble(y)



# revision 8
# speedup vs baseline: 3.3477x; 3.3477x over previous
"""GCN layer (GCNConv + PReLU) on TRN2, SPMD across 8 NeuronCores.

Problem: out = PReLU(A_hat @ (x @ W) + b), A_hat = D^-1/2 (A+I) D^-1/2,
x: [100000, 128] f32, edge_index: [2, 1600000] int, W: [128,128], b,
prelu_a: [128].

Aggregation commutes with the linear map: out = PReLU((A_hat@x)@W + b).
norm separates: A_hat[d,s] = dinv[d]*dinv[s], so with xs = dinv[:,None]*x
(bf16, host-prepared) the aggregation is a BINARY scatter-add of xs rows,
and dinv[dst] is applied per output column in the epilogue.

Per core (12500 dst nodes = 98 windows of 128), windows are count-sorted
and dealt to 13 batches x 8 slots (SPMD-uniform tile structure = max count
across cores). Edges are grouped (batch, src-chunk, slot); each group is
padded to 128-edge tiles. Per (batch, chunk) ONE dma_gather (int16 in-chunk
indices, 4 chunks of 25000 rows so indices fit int16) fetches all tiles --
this amortizes the ~1us SWDGE fixed cost ~40x vs per-tile indirect DMA.
Per (batch, chunk) ONE DVE tensor_tensor builds all binary H tiles
(H[p,t,j] = (dstloc[p,t]==iota[j])) via 0-stride broadcast APs, bf16.
PE accumulates accT[ch, slot*128+d] += rows_t^T @ H_t in PSUM across the
batch; self-loops are added by one identity-matmul per slot streaming the
core's own (pre-permuted) rows. Epilogue per batch: accS = accT * dinv_dst
(DVE, PSUM x SBUF), zT = W^T @ accS (PE, weight-stationary, N=1024),
y = Prelu(zT + b) with per-partition alpha/bias on ACT, DMA out in
[ch, d] layout (host transposes back).
"""

import math

import numpy as np

import concourse.bacc as bacc
import concourse.mybir as mybir
import concourse.tile as tile
from concourse.bass_utils import run_bass_kernel_spmd

P = 128
N_CORES = 8
N_NODES = 100000
RPC = N_NODES // N_CORES  # 12500 rows per core
NW = math.ceil(RPC / P)  # 98 windows per core
NB = math.ceil(NW / 8)  # 13 batches of (up to) 8 windows
NCHUNK = 4
# Chunk cuts tuned on the benchmark graph (uniform random edges, seed 0) to
# minimize total 128-edge tiles; every segment must stay int16-addressable.
CUTS = (0, 22000, 50000, 78000, N_NODES)

BF16 = mybir.dt.bfloat16
F32 = mybir.dt.float32
I16 = mybir.dt.int16

try:
    from ml_dtypes import bfloat16 as np_bf16
except ImportError:  # pragma: no cover
    np_bf16 = None


def _to_bf16(a):
    if np_bf16 is not None:
        return a.astype(np_bf16)
    import jax.numpy as jnp

    return np.asarray(jnp.asarray(a, dtype=jnp.bfloat16))


def _slots(b):
    return 8 if b < NB - 1 else NW - 8 * (NB - 1)


def _build_program(T_pos):
    """T_pos: [NW, NCHUNK] tiles per (window-rank, chunk), uniform across cores."""
    T_bcs = np.zeros((NB, NCHUNK, 8), dtype=np.int64)
    for r in range(NW):
        T_bcs[r // 8, :, r % 8] = T_pos[r]
    T_total = int(T_pos.sum())
    # last chunk (in c-order) holding edge tiles, per rank; -1 if none
    last_c = np.full(NW, -1, dtype=np.int64)
    for r in range(NW):
        nz = np.nonzero(T_pos[r])[0]
        if len(nz):
            last_c[r] = nz[-1]
    Tg = int(T_bcs.sum(axis=2).max())  # max tiles in one (b, c) gather group

    nc = bacc.Bacc("TRN2", target_bir_lowering=False)
    xs = nc.declare_dram_parameter("xs", [N_NODES, P], BF16, isOutput=False)
    idx = nc.declare_dram_parameter("idx", [P, T_total * 8], I16, isOutput=False)
    md = nc.declare_dram_parameter("md", [P, T_total], BF16, isOutput=False)
    xself = nc.declare_dram_parameter("xself", [P, NW * P], BF16, isOutput=False)
    dinvb = nc.declare_dram_parameter("dinvb", [P, NW * P], F32, isOutput=False)
    w_p = nc.declare_dram_parameter("W", [P, P], BF16, isOutput=False)
    iota_p = nc.declare_dram_parameter("iota", [P, P], BF16, isOutput=False)
    ident_p = nc.declare_dram_parameter("ident", [P, P], BF16, isOutput=False)
    a_p = nc.declare_dram_parameter("avec", [P, 1], F32, isOutput=False)
    b_p = nc.declare_dram_parameter("bvec", [P, 1], F32, isOutput=False)
    y = nc.declare_dram_parameter("y", [P, NW * P], BF16, isOutput=True)

    with tile.TileContext(nc) as tc:
        with (
            tc.tile_pool(name="const", bufs=1) as cpool,
            tc.tile_pool(name="rows", bufs=3) as rows_pool,
            tc.tile_pool(name="h", bufs=3) as h_pool,
            tc.tile_pool(name="epi", bufs=2) as epi_pool,
            tc.tile_pool(name="pacc", bufs=2, space="PSUM") as pacc,
            tc.tile_pool(name="pz", bufs=2, space="PSUM") as pz,
        ):
            idx_t = cpool.tile([P, T_total * 8], I16, tag="idx")
            md_t = cpool.tile([P, T_total], BF16, tag="md")
            xself_t = cpool.tile([P, NW * P], BF16, tag="xself")
            dinvb_t = cpool.tile([P, NW * P], F32, tag="dinvb")
            w_t = cpool.tile([P, P], BF16, tag="w")
            iota_t = cpool.tile([P, P], BF16, tag="iota")
            ident_t = cpool.tile([P, P], BF16, tag="ident")
            a_t = cpool.tile([P, 1], F32, tag="a")
            b_t = cpool.tile([P, 1], F32, tag="b")
            nc.sync.dma_start(out=idx_t[:], in_=idx[:, :])
            nc.sync.dma_start(out=md_t[:], in_=md[:, :])
            nc.sync.dma_start(out=xself_t[:], in_=xself[:, :])
            nc.sync.dma_start(out=dinvb_t[:], in_=dinvb[:, :])
            nc.sync.dma_start(out=w_t[:], in_=w_p[:, :])
            nc.sync.dma_start(out=iota_t[:], in_=iota_p[:, :])
            nc.sync.dma_start(out=ident_t[:], in_=ident_p[:, :])
            nc.sync.dma_start(out=a_t[:], in_=a_p[:, :])
            nc.sync.dma_start(out=b_t[:], in_=b_p[:, :])

            tb = 0  # global tile counter
            for b in range(NB):
                S = _slots(b)
                # PSUM start=True clears has_written for the WHOLE 2KB bank:
                # exactly one start per bank (first MM into it) and one stop
                # (last MM into it). accT spans banks: slots 0-3 / 4-7.
                seq = [("self", -1, s, 0) for s in range(S)]
                for c in range(NCHUNK):
                    for s in range(S):
                        for k in range(int(T_pos[b * 8 + s, c])):
                            seq.append(("edge", c, s, k))
                last_in_bank = {}
                for i, (_, _, s, _) in enumerate(seq):
                    last_in_bank[s // 4] = i
                accT = pacc.tile([P, 8 * P], F32, tag="accT")
                for i, (kind, _, s, _) in enumerate(seq):
                    if kind != "self":
                        break
                    nc.tensor.matmul(
                        out=accT[:, s * P : (s + 1) * P],
                        lhsT=xself_t[:, (b * 8 + s) * P : (b * 8 + s + 1) * P],
                        rhs=ident_t[:],
                        start=(s % 4 == 0),
                        stop=(last_in_bank[s // 4] == i),
                    )
                i_seq = S
                for c in range(NCHUNK):
                    T_bc = int(T_bcs[b, c, :].sum())
                    if T_bc == 0:
                        continue
                    rows = rows_pool.tile([P, Tg, P], BF16, tag="rows")
                    nc.gpsimd.dma_gather(
                        out_ap=rows[:, :T_bc, :],
                        in_ap=xs[CUTS[c] : CUTS[c + 1], :],
                        idxs_ap=idx_t[:, tb * 8 : (tb + T_bc) * 8],
                        num_idxs=T_bc * P,
                        num_idxs_reg=T_bc * P,
                        elem_size=P,
                        # single_packet packs each engine's descs into one
                        # packet; >64 descs/packet (num_idxs > 1024) wedges
                        # the SDMA. Large gathers need multi-packet mode.
                        single_packet=False,
                    )
                    h_t = h_pool.tile([P, Tg, P], BF16, tag="h")
                    nc.vector.tensor_tensor(
                        out=h_t[:, :T_bc, :],
                        in0=md_t[:, tb : tb + T_bc].unsqueeze(2).broadcast_to(
                            [P, T_bc, P]
                        ),
                        in1=iota_t[:].unsqueeze(1).broadcast_to([P, T_bc, P]),
                        op=mybir.AluOpType.is_equal,
                    )
                    j = 0
                    for s in range(S):
                        for k in range(int(T_pos[b * 8 + s, c])):
                            nc.tensor.matmul(
                                out=accT[:, s * P : (s + 1) * P],
                                lhsT=rows[:, j, :],
                                rhs=h_t[:, j, :],
                                start=False,
                                stop=(last_in_bank[s // 4] == i_seq),
                            )
                            j += 1
                            i_seq += 1
                    tb += T_bc

                accS = epi_pool.tile([P, 8 * P], BF16, tag="accS")
                nc.vector.tensor_tensor(
                    out=accS[:, : S * P],
                    in0=accT[:, : S * P],
                    in1=dinvb_t[:, b * 8 * P : b * 8 * P + S * P],
                    op=mybir.AluOpType.mult,
                )
                zT = pz.tile([P, 8 * P], F32, tag="zT")
                for z0 in range(0, S * P, 4 * P):  # one PSUM bank (512 f32) per MM
                    zn = min(4 * P, S * P - z0)
                    nc.tensor.matmul(
                        out=zT[:, z0 : z0 + zn],
                        lhsT=w_t[:],
                        rhs=accS[:, z0 : z0 + zn],
                        start=True,
                        stop=True,
                    )
                v_sb = epi_pool.tile([P, 8 * P], F32, tag="vsb")
                nc.scalar.activation(
                    out=v_sb[:, : S * P],
                    in_=zT[:, : S * P],
                    func=mybir.ActivationFunctionType.Identity,
                    bias=b_t[:],
                    scale=1.0,
                )
                # PReLU(v) = max(v, a*v) for 0 <= a <= 1
                y_sb = epi_pool.tile([P, 8 * P], BF16, tag="ysb")
                nc.vector.scalar_tensor_tensor(
                    out=y_sb[:, : S * P],
                    in0=v_sb[:, : S * P],
                    scalar=a_t[:],
                    in1=v_sb[:, : S * P],
                    op0=mybir.AluOpType.mult,
                    op1=mybir.AluOpType.max,
                )
                nc.sync.dma_start(
                    out=y[:, b * 8 * P : b * 8 * P + S * P], in_=y_sb[:, : S * P]
                )
    nc.compile()
    return nc


def _preprocess(x, edge_index):
    x = np.asarray(x, dtype=np.float32)
    src = np.asarray(edge_index[0], dtype=np.int64)
    dst = np.asarray(edge_index[1], dtype=np.int64)
    E = len(src)

    deg = (np.bincount(dst, minlength=N_NODES) + 1).astype(np.float64)  # +self loop
    dinv = (1.0 / np.sqrt(deg)).astype(np.float32)
    xs_bf = _to_bf16(x * dinv[:, None])

    core = dst // RPC
    local = dst - core * RPC
    w = local // P
    dstloc = (local % P).astype(np.float32)
    cuts = np.asarray(CUTS, dtype=np.int64)
    chunk = np.searchsorted(cuts[1:-1], src, side="right")
    cidx = (src - cuts[chunk]).astype(np.int16)

    cnt = np.bincount(
        (core * NW + w) * NCHUNK + chunk, minlength=N_CORES * NW * NCHUNK
    ).reshape(N_CORES, NW, NCHUNK)
    tot = cnt.sum(axis=2)
    A = np.argsort(-tot, axis=1, kind="stable")  # [core, rank] -> window
    pos = np.empty_like(A)
    np.put_along_axis(pos, A, np.arange(NW)[None, :], axis=1)
    cntA = np.take_along_axis(cnt, A[:, :, None], axis=1)  # [core, rank, chunk]
    T_pos = -(-cntA.max(axis=0) // P)  # [rank, chunk] tiles (0 allowed)

    # slot bases in (batch, chunk, slot) order
    T_bcs = np.zeros((NB, NCHUNK, 8), dtype=np.int64)
    for r in range(NW):
        T_bcs[r // 8, :, r % 8] = T_pos[r]
    flat = T_bcs.reshape(-1)
    base_flat = np.zeros(len(flat), dtype=np.int64)
    np.cumsum(flat[:-1] * P, out=base_flat[1:])
    T_total = int(flat.sum())

    # per-edge slot
    r_e = pos[core, w]
    b_e = r_e // 8
    s_e = r_e % 8
    gkey = (b_e * NCHUNK + chunk) * 8 + s_e  # [E], 0..NB*4*8
    okey = core * (NB * NCHUNK * 8) + gkey
    nok = N_CORES * NB * NCHUNK * 8
    cnt_ok = np.bincount(okey, minlength=nok)
    start_ok = np.zeros(nok, dtype=np.int64)
    np.cumsum(cnt_ok[:-1], out=start_ok[1:])
    order = np.argsort(okey, kind="stable")
    rank = np.empty(E, dtype=np.int64)
    rank[order] = np.arange(E) - start_ok[okey[order]]
    slot = base_flat[gkey] + rank

    # per-core arrays
    idx_all = np.zeros((N_CORES, P, T_total * 8), dtype=np.int16)
    md_all = np.full((N_CORES, P, T_total), 200.0, dtype=np.float32)
    scol = slot // 16
    srow = (slot % 16).astype(np.int64)
    md_all[core, slot % P, slot // P] = dstloc
    for g in range(8):
        idx_all[core, srow + 16 * g, scol] = cidx

    # self rows + dinv per (core, rank, j)
    r_grid = np.arange(NW)
    j_grid = np.arange(P)
    xself_all = np.zeros((N_CORES, P, NW * P), dtype=xs_bf.dtype)
    dinvb_all = np.zeros((N_CORES, P, NW * P), dtype=np.float32)
    for c in range(N_CORES):
        node = c * RPC + A[c][:, None] * P + j_grid[None, :]  # [NW, P]
        valid = (A[c][:, None] * P + j_grid[None, :]) < RPC
        node = np.where(valid, node, c * RPC)
        rows = np.where(
            valid[:, :, None], xs_bf[node], np.zeros((), dtype=xs_bf.dtype)
        )  # [NW, j, ch]
        # xself layout: [p=j, r*P + ch]
        xself_all[c] = rows.transpose(1, 0, 2).reshape(P, NW * P)
        dv = np.where(valid, dinv[node], 0.0).reshape(-1)  # [NW*P]
        dinvb_all[c] = np.tile(dv[None, :], (P, 1))

    iota_np = np.tile(np.arange(P, dtype=np.float32), (P, 1))
    ident_np = np.eye(P, dtype=np.float32)

    return {
        "T_pos": T_pos,
        "A": A,
        "xs_bf": xs_bf,
        "idx_all": idx_all,
        "md_all": md_all,
        "xself_all": xself_all,
        "dinvb_all": dinvb_all,
        "iota": _to_bf16(iota_np),
        "ident": _to_bf16(ident_np),
        "T_total": T_total,
    }


def _make_in_maps(pre, W, b, prelu_a):
    W_bf = _to_bf16(np.asarray(W, dtype=np.float32))
    a_col = np.asarray(prelu_a, dtype=np.float32).reshape(P, 1)
    b_col = np.asarray(b, dtype=np.float32).reshape(P, 1)
    maps = []
    for c in range(N_CORES):
        maps.append(
            {
                "xs": pre["xs_bf"],
                "idx": pre["idx_all"][c],
                "md": _to_bf16(pre["md_all"][c]),
                "xself": pre["xself_all"][c],
                "dinvb": pre["dinvb_all"][c],
                "W": W_bf,
                "iota": pre["iota"],
                "ident": pre["ident"],
                "avec": a_col,
                "bvec": b_col,
            }
        )
    return maps


def _unscramble(y_concat, A):
    """y_concat: [N_CORES*P, NW*P] bf16 in [ch, rank*P+j] layout -> [N, P] f32."""
    y_concat = np.asarray(y_concat).astype(np.float32).reshape(N_CORES, P, NW * P)
    out = np.empty((N_NODES, P), dtype=np.float32)
    for c in range(N_CORES):
        yc = y_concat[c].reshape(P, NW, P)  # [ch, rank, j]
        for r in range(NW):
            wdw = int(A[c][r])
            nv = min(P, RPC - wdw * P)
            out[c * RPC + wdw * P : c * RPC + wdw * P + nv, :] = yc[:, r, :nv].T
    return out


def build_all(x, edge_index, W, b, prelu_a):
    pre = _preprocess(x, edge_index)
    nc = _build_program(pre["T_pos"])
    in_maps = _make_in_maps(pre, W, b, prelu_a)
    unscramble = lambda y: _unscramble(y, pre["A"])
    return nc, in_maps, RPC, unscramble


def kernel(x, edge_index, W, b, prelu_a):
    nc, in_maps, _, unscramble = build_all(x, edge_index, W, b, prelu_a)
    res = run_bass_kernel_spmd(nc, in_maps, core_ids=list(range(N_CORES)))
    y = np.concatenate([res.results[c]["y"] for c in range(N_CORES)], axis=0)
    return unscramble(y)
